# revision 30
# baseline (speedup 1.0000x reference)
"""ExpertConv2d Trainium2 kernel: per-patch mixture-of-experts 3x3 conv.

Problem: x (4,64,512,512) f32 split into 256 patches of (64ch, 64x64);
each patch convolved (pad=1) with a per-patch mix of 5 expert kernels
(mix weights v), plus mixed bias.  Data-parallel over patches across 8
NeuronCores (32 patches/core, processed as 16 patch-pairs).

Device plan per core:
 - mixing: agg_w[p, (ci,t,co)] = sum_k vv[p,k] * W_flat[k, (ci,t,co)]
   via K=5 matmuls (bf16, fp32 psum), cast to bf16, redistributed into
   per-pair weight tiles [128, 9*64] (patch A on partitions 0-63, B on
   64-127) with SB->SB DMAs.
 - conv: per pair, x tile [128, 4096] bf16 (A | B channel blocks; cast
   fp32->bf16 inline in the gpsimd DMA).  Per chunk (8 y-rows = 512
   outputs) 9 tap-matmuls accumulate in PSUM; boundary taps shrink the
   output rectangle (padding handled by geometry, not memsets).
   Quadrants: row group = patch half, col group (psum half) = patch ^
   chunk parity, so 4 K=64/M=64 matmuls run concurrently = full PE.
 - copyback: ACT/DVE per-partition bias add PSUM->SBUF f32, then one
   contiguous 2MB DMA out per pair.  Host unscrambles the layout.
"""

import os
import sys

import numpy as np

sys.path.insert(0, "/opt/trn_rl_repo")

import concourse.bass as bass  # noqa: E402
import concourse.tile as tile  # noqa: E402
from concourse import mybir  # noqa: E402

import bass_rust as _bass_rust  # noqa: E402

# ---------------------------------------------------------------------------
# Workaround: this walrus build rejects >2 sync-waits on one instruction.
# TileContext._drain_and_barrier attaches one wait per live sem lane to a
# single SP Drain.  Replace it: one SP wait_ge per lane, then a clean drain.
# ---------------------------------------------------------------------------


def _split_drain_and_barrier(self, tick_clock, wait_clock):
    nc = self.nc
    gc = tick_clock.global_clock
    assert self.sems is not None
    allocated = self.sems.allocated()
    for proc, sem in sorted(allocated.items()):
        t = gc[proc] if proc < len(gc) else 0
        if t > 0:
            nc.sync.wait_ge(sem, _bass_rust.tick_to_sem(t, proc))
    nc.sync.drain()
    nc.all_engine_barrier()
    popped = nc._tile_sem_poison_stack.pop()
    assert popped is self._sem_poison
    nc.clear_and_free_semaphores(list(allocated.values()))
    nc.all_engine_barrier()


tile.TileContext._drain_and_barrier = _split_drain_and_barrier

_MAX_WAITS = 1


def _split_excess_waits(nc):
    """Walrus (CoreV2/V3 setupSyncWait) accepts at most 2 sem-waits per
    instruction.  Tile can attach more.  Move the excess onto NoOps inserted
    immediately before the instruction on the same engine (same queue order,
    so semantics are unchanged)."""
    n_split = 0
    for fn in nc.m.functions:
        for bb in fn.blocks:
            insts = list(bb.instructions)
            out = []
            changed = False
            for inst in insts:
                si = inst.sync_info
                waits = list(si.on_wait) if si is not None and si.on_wait else []
                if len(waits) > _MAX_WAITS:
                    keep = waits[-_MAX_WAITS:]
                    excess = waits[:-_MAX_WAITS]
                    for i in range(0, len(excess), _MAX_WAITS):
                        grp = excess[i:i + _MAX_WAITS]
                        nop = mybir.InstNoOp(
                            name=f"{inst.name}_wsplit{i}", ins=[], outs=[])
                        nop.engine = inst.engine
                        nop.sync_info = mybir.SyncInfo(on_wait=grp, on_update=[])
                        out.append(nop)
                    inst.sync_info = mybir.SyncInfo(
                        on_wait=keep,
                        on_update=list(si.on_update) if si.on_update else [])
                    changed = True
                    n_split += 1
                out.append(inst)
            if changed:
                bb.instructions = out
    return n_split

def _strip_reuse_ldweights(nc, reuse_names):
    """Remove the InstLdweights paired with matmuls whose stationary operand
    is already loaded in their PE quadrant (same patch+tap weights loaded two
    instructions earlier for the sibling chunk).  Merges the ldweights' sync
    info into the matmul so no ordering edges are lost."""
    n = 0
    for fn in nc.m.functions:
        for bb in fn.blocks:
            insts = list(bb.instructions)
            out = []
            changed = False
            k = 0
            while k < len(insts):
                inst = insts[k]
                nxt = insts[k + 1] if k + 1 < len(insts) else None
                if (isinstance(inst, mybir.InstLdweights)
                        and nxt is not None
                        and isinstance(nxt, mybir.InstMatmult)
                        and nxt.name in reuse_names):
                    lsi = inst.sync_info
                    if lsi is not None and (lsi.on_wait or lsi.on_update):
                        msi = nxt.sync_info
                        mw = list(msi.on_wait) if msi and msi.on_wait else []
                        mu = list(msi.on_update) if msi and msi.on_update else []
                        nxt.sync_info = mybir.SyncInfo(
                            on_wait=list(lsi.on_wait or []) + mw,
                            on_update=mu + list(lsi.on_update or []))
                    changed = True
                    n += 1
                    k += 1
                    continue
                out.append(inst)
                k += 1
            if changed:
                bb.instructions = out
    return n


# ---------------------------------------------------------------------------
# Constants (hardcoded problem shape)
# ---------------------------------------------------------------------------
B, C_IN, C_OUT, K, KS, P_SZ, HW = 4, 64, 64, 5, 3, 64, 512
GRID = HW // P_SZ                  # 8x8 patch grid
N_CORES = 8
N_PATCH = B * GRID * GRID          # 256
PPC = N_PATCH // N_CORES           # 32 patches per core
NPAIR = PPC // 2                   # 16 pairs per core
NCHUNK = 8                         # 512-wide output chunks per patch
WFREE = C_IN * KS * KS * C_OUT     # 36864  (ci, t, co) flat weight size
BF16 = mybir.dt.bfloat16
F32 = mybir.dt.float32

_NC_CACHE = {}
_REUSE_MM_NAMES = set()


def _tap_geometry(c, ky, kx):
    """Output sub-rectangle of chunk c covered by tap (ky, kx) and the
    matching input offset.  Returns None if empty (never happens here)."""
    y0 = max(8 * c, 1 - ky)
    y1 = min(8 * c + 8, P_SZ + 1 - ky)
    x0 = max(0, 1 - kx)
    x1 = min(P_SZ, P_SZ + 1 - kx)
    if y0 >= y1 or x0 >= x1:
        return None
    in_off = (y0 + ky - 1) * P_SZ + (x0 + kx - 1)
    out_off = (y0 - 8 * c) * P_SZ + x0
    return in_off, out_off, y1 - y0, x1 - x0


def build_nc(npair=NPAIR, split_waits=True):
    nc = bass.Bass("TRN2")
    xin = nc.dram_tensor("xin", [npair, 128, P_SZ * P_SZ], BF16, kind="ExternalInput")
    wflat = nc.dram_tensor("wflat", [K, WFREE], BF16, kind="ExternalInput")
    vvlo = nc.dram_tensor("vvlo", [K, 2 * npair], BF16, kind="ExternalInput")
    vvhi = nc.dram_tensor("vvhi", [K, 2 * npair], BF16, kind="ExternalInput")
    bbank = nc.dram_tensor("bbank", [K, C_OUT], BF16, kind="ExternalInput")
    out = nc.dram_tensor("out", [npair, 128, P_SZ * P_SZ], BF16, kind="ExternalOutput")

    npatch = 2 * npair
    with tile.TileContext(nc) as tc:
        with (
            tc.tile_pool(name="persist", bufs=1) as persist,
            tc.tile_pool(name="xpool", bufs=6) as xpool,
            tc.tile_pool(name="opool", bufs=4) as opool,
            tc.tile_pool(name="mixw", bufs=2) as mixw,
            tc.tile_pool(name="psum", bufs=8, space="PSUM") as pp,
        ):
            # ---- small constants ----
            vvlo_sb = persist.tile([K, npatch], BF16)
            nc.sync.dma_start(out=vvlo_sb, in_=vvlo[:, :])
            vvhi_sb = persist.tile([K, npatch], BF16)
            nc.sync.dma_start(out=vvhi_sb, in_=vvhi[:, :])
            bbank_sb = persist.tile([K, C_OUT], BF16)
            nc.sync.dma_start(out=bbank_sb, in_=bbank[:, :])

            # ---- weight mixing: agg[p, f] = sum_k vv[p,k] wflat[k,f] ----
            agg_sb = persist.tile([npatch, WFREE], BF16)
            NCH = 512
            piece_sz = WFREE // 8
            for piece in range(8):
                wf_sb = mixw.tile([K, piece_sz], BF16, tag="wf")
                nc.sync.dma_start(out=wf_sb,
                                  in_=wflat[:, piece * piece_sz:(piece + 1) * piece_sz])
                for c in range(piece_sz // NCH):
                    f0 = c * NCH
                    psum_m = pp.tile([npatch, NCH], F32, tag="pc", name="psum_m")
                    nc.tensor.matmul(psum_m, lhsT=vvlo_sb, rhs=wf_sb[:, f0:f0 + NCH],
                                     start=True, stop=True)
                    dst = agg_sb[:, piece * piece_sz + f0: piece * piece_sz + f0 + NCH]
                    if c % 9 < 4:
                        nc.scalar.copy(out=dst, in_=psum_m)
                    else:
                        nc.vector.tensor_copy(dst, psum_m)

            # ---- bias mixing:  bias_sb[0:64, p] = bias(patch p),
            #      bias_sb[64:128, p] = bias(pair-swapped p) ----
            psum_b = pp.tile([128, npatch], F32, tag="pc", name="psum_b")
            nc.tensor.matmul(psum_b[0:64, :], lhsT=bbank_sb, rhs=vvlo_sb,
                             start=True, stop=True, skip_group_check=True)
            nc.tensor.matmul(psum_b[64:128, :], lhsT=bbank_sb, rhs=vvhi_sb,
                             start=True, stop=True, skip_group_check=True)
            bias_sb = persist.tile([128, npatch], F32)
            nc.scalar.copy(out=bias_sb, in_=psum_b)

            # ---- per-pair weight tiles: [128, 9*64] bf16;
            #      partitions 0-63 patch A taps, 64-127 patch B ----
            w_all = persist.tile([128, npair, KS * KS * C_OUT], BF16)
            dma_engs = (nc.gpsimd, nc.gpsimd)
            for p in range(npatch):
                j, hp = p // 2, p % 2
                src = agg_sb[p:p + 1, :].rearrange("q (ci f) -> q ci f", ci=C_IN)
                eng = dma_engs[p % 2]
                eng.dma_start(out=w_all[64 * hp:64 * hp + 64, j, :], in_=src)

            # ---- main loop over pairs ----
            taps = [(1, 1)] + [(ky, kx) for ky in range(KS) for kx in range(KS)
                               if (ky, kx) != (1, 1)]
            for j in range(npair):
                x_t = xpool.tile([128, P_SZ * P_SZ], BF16, tag="x")
                nc.sync.dma_start(out=x_t, in_=xin[j, :, :])
                o_t = opool.tile([128, P_SZ * P_SZ], BF16, tag="o")

                for c4 in range(NCHUNK // 4):
                    chunks = tuple(4 * c4 + i for i in range(4))
                    psums = {}
                    for c in chunks:
                        psums[c] = pp.tile([128, 512], F32, tag="pc", name="pc")
                    for ti, (ky, kx) in enumerate(taps):
                        first = ti == 0
                        last = ti == len(taps) - 1
                        # Order so the 4 in-flight matmuls cover 4 distinct
                        # PSUM banks and all 4 PE quadrants; second wave reuses
                        # each quadrant's already-loaded weights.
                        order = [(chunks[0], 0, False), (chunks[1], 0, False),
                                 (chunks[2], 1, False), (chunks[3], 1, False),
                                 (chunks[2], 0, True), (chunks[3], 0, True),
                                 (chunks[0], 1, True), (chunks[1], 1, True)]
                        for c, P, reuse in order:
                            if True:
                                h = P ^ (c & 1)
                                in_off, out_off, cy, cx = _tap_geometry(c, ky, kx)
                                y_in0 = in_off // P_SZ
                                x_in0 = in_off % P_SZ
                                rhs = x_t[64 * P:64 * P + 64, :].rearrange(
                                    "p (y x) -> p y x", x=P_SZ)[
                                    :, y_in0:y_in0 + cy, x_in0:x_in0 + cx]
                                y_o0 = out_off // P_SZ
                                x_o0 = out_off % P_SZ
                                outap = psums[c][64 * h:64 * h + 64, :].rearrange(
                                    "p (y x) -> p y x", x=P_SZ)[
                                    :, y_o0:y_o0 + cy, x_o0:x_o0 + cx]
                                t = ky * KS + kx
                                lhsT = w_all[64 * P:64 * P + 64, j,
                                             t * C_OUT:(t + 1) * C_OUT]
                                mi = nc.tensor.matmul(outap, lhsT=lhsT, rhs=rhs,
                                                      start=first, stop=last,
                                                      skip_group_check=True)
                                if reuse:
                                    _REUSE_MM_NAMES.add(mi.ins.name)
                    for c in chunks:
                        col = 2 * j + (c & 1)
                        dst = o_t[:, c * 512:(c + 1) * 512]
                        if c & 1:
                            nc.vector.tensor_scalar_add(
                                dst, psums[c], bias_sb[:, col:col + 1])
                        else:
                            nc.scalar.activation(
                                dst, psums[c],
                                mybir.ActivationFunctionType.Identity,
                                bias=bias_sb[:, col:col + 1], scale=1.0)
                if j < npair - 1:
                    half_f = P_SZ * P_SZ // 2
                    nc.sync.dma_start(out=out[j, :, 0:half_f], in_=o_t[:, 0:half_f])
                    nc.scalar.dma_start(out=out[j, :, half_f:], in_=o_t[:, half_f:])
                else:
                    qf = P_SZ * P_SZ // 4
                    for q in range(4):
                        eng = nc.sync if q % 2 == 0 else nc.scalar
                        eng.dma_start(out=out[j, :, q * qf:(q + 1) * qf],
                                      in_=o_t[:, q * qf:(q + 1) * qf])
    ns = _strip_reuse_ldweights(nc, _REUSE_MM_NAMES)
    if split_waits:
        n = _split_excess_waits(nc)
        if n:
            print(f"[kernel] split {n} waits; stripped {ns} ldweights")
    return nc


# ---------------------------------------------------------------------------
# Host marshalling
# ---------------------------------------------------------------------------


def _marshal_inputs(x, v, weight, bias):
    import ml_dtypes

    bf16 = ml_dtypes.bfloat16
    # x: (B, C, 512, 512) -> per patch (b, gy, gx) blocks of [64, 64, 64]
    xp = x.reshape(B, C_IN, GRID, P_SZ, GRID, P_SZ)
    xp = xp.transpose(0, 2, 4, 1, 3, 5)          # b gy gx ci y x
    xp = np.ascontiguousarray(xp).reshape(N_PATCH, C_IN, P_SZ * P_SZ)
    # per core: [NPAIR, 128(=2 patches x ci), 4096]
    xc = xp.reshape(N_CORES, NPAIR, 2 * C_IN, P_SZ * P_SZ)

    # vv: (b, k, gy, gx) -> [patch, k]
    vv = v.transpose(0, 2, 3, 1).reshape(N_PATCH, K)
    vvc = vv.reshape(N_CORES, PPC, K)
    vv_lo = vvc.transpose(0, 2, 1).astype(bf16)              # [core, K, 32]
    swap = vvc.reshape(N_CORES, NPAIR, 2, K)[:, :, ::-1, :]
    vv_hi = swap.reshape(N_CORES, PPC, K).transpose(0, 2, 1).astype(bf16)

    # weight: (k, co, ci, ky, kx) -> [k, (ci, t, co)]
    wf = weight.transpose(0, 2, 3, 4, 1).reshape(K, C_IN, KS * KS, C_OUT)
    wf = np.ascontiguousarray(wf).reshape(K, WFREE).astype(bf16)

    bb = bias.astype(bf16)                                   # [k, co]

    in_maps = []
    for m in range(N_CORES):
        in_maps.append({
            "xin": np.ascontiguousarray(xc[m]).astype(bf16),
            "wflat": wf,
            "vvlo": np.ascontiguousarray(vv_lo[m]),
            "vvhi": np.ascontiguousarray(vv_hi[m]),
            "bbank": bb,
        })
    return in_maps


def _unmarshal_output(dev_outs):
    """dev_outs: list of 8 arrays [NPAIR, 128, 4096] f32 -> (B, C_OUT, 512, 512)."""
    out = np.empty((B, C_OUT, HW, HW), np.float32)
    patches = np.empty((N_PATCH, C_OUT, P_SZ, P_SZ), np.float32)
    for m in range(N_CORES):
        a = dev_outs[m].astype(np.float32).reshape(NPAIR, 2, C_OUT, 4, 2, 8, P_SZ)
        # axes: j, h, co, c2, cp, yy, x ; patch_local = h ^ cp
        p0 = a[:, :, :, :, 0]                      # cp=0: patch = h
        p1 = a[:, ::-1, :, :, 1]                   # cp=1: patch = 1-h
        b = np.stack([p0, p1], axis=4)             # j, patch, co, c2, cp, yy, x
        b = b.reshape(NPAIR, 2, C_OUT, NCHUNK * 8, P_SZ)  # y = (c2, cp, yy)
        patches[m * PPC:(m + 1) * PPC] = b.reshape(PPC, C_OUT, P_SZ, P_SZ)
    pt = patches.reshape(B, GRID, GRID, C_OUT, P_SZ, P_SZ)
    out = pt.transpose(0, 3, 1, 4, 2, 5).reshape(B, C_OUT, HW, HW)
    return np.ascontiguousarray(out)


def kernel(x, v, weight, bias, trace=False):
    from concourse.bass_utils import run_bass_kernel_spmd

    x = np.asarray(x, dtype=np.float32)
    v = np.asarray(v, dtype=np.float32)
    weight = np.asarray(weight, dtype=np.float32)
    bias = np.asarray(bias, dtype=np.float32)

    if "nc" not in _NC_CACHE:
        _NC_CACHE["nc"] = build_nc()
    nc = _NC_CACHE["nc"]

    in_maps = _marshal_inputs(x, v, weight, bias)
    res = run_bass_kernel_spmd(nc, in_maps, core_ids=list(range(N_CORES)),
                               trace=trace)
    dev_outs = [res.results[m]["out"] for m in range(N_CORES)]
    full = _unmarshal_output(dev_outs)
    kernel.last_result = res
    return full
